# revision 36
# baseline (speedup 1.0000x reference)
"""GQA kernel for Trainium2 (Bass/Tile), 8-core head-parallel. v4.

Problem: x(1,2048,1024), Wq(1024,1024)+bq, Wk/Wv(1024,256)+bk/bv,
16 Q heads / 4 KV heads, head_dim 64, full (non-causal) softmax attention.
Reference output is attn(B,H,S,Dh) reshaped DIRECTLY to (B,S,H*Dh):
out rows [h*128,(h+1)*128) of the (2048,1024) output belong to head h.

Sharding: core d owns Q heads {2d, 2d+1} (both share KV head d//2), so each
core computes a contiguous (256,1024) slab of the final output.

Host-side prep (free): x transposed+cast to bf16 xT (1024,2048); per-core
weight slices pre-scaled (Wq/8 folds 1/sqrt(64)) and packed Wkv=[Wk|Wv],
all cast to bf16.

v4 structure (v2 baseline 113.6-115us; v4 measures ~112-113us):
  - Engine budget (measured): scalar exp 71us busy (64 ACTIVATEs of 1024
    cols, ~1110ns each), PE union ~85us, DVE ~24us. Both scalar AND PE are
    near-saturated inside the exp window -> all PV/output work is load-
    balanced across the whole exp span via a deferred-work fifo.
  - Timeline: first ACTIVATE ~24-25us (floor: 3-queue DMA lands wkv+wq+xT
    block0 ~15-17us, then kv-quarter1 + q chain at MID pstate), exp gaps
    ~10-12us (all in the q0/wave era), tail ~9us.
  - HAM pstate: PE runs 1.2GHz until ~12-17us of near-continuous activity
    (ham k=8/8 at t~20-26us); idle gaps reset/delay the ramp and can cause
    mid-kernel downclock. Zero-dependency warmup (garbage SBUF weights)
    starts the ramp at ~6.7us. Longer warmup chains DELAY real work ~1:1
    (the Tile DAG scheduler prefers earlier-emitted ready work), so keep
    warmup short.
  - DMA queues (measured concurrent): gpsimd ~100 GB/s, sync ~45-70,
    scalar ~68; the DVE cannot issue DMAs. dma_start only ISSUES; data
    starts ~2.5us later. wkv split across gpsimd+sync first, wq on scalar
    early, xT strips balanced by rate (sync gets only 2-chunk strips).
  - B(0) kv projection in kb-column-quarters so kb0's K is ready before
    the full 512-col q chain finishes.
  - ALL PVs are deferred into a fifo of ("pv", pso, pt, kb, ready_seq) and
    ("out", qb, pso) items, drained <=2 pops per exp slot in the q1..q3
    eras (more when backlogged, none during the DMA-paced q0 waves). Pops
    require the exp >=1 slot old, else the PE stalls on the activation.
    fifo order serializes pso psum-bank reuse across eras (bufs=1 ring).
  - pt ring bufs=20 (~5MB SBUF) holds the deferred exp outputs.
  - PV stays two 512-col MMs per kb (ISA caps matmul moving size; a merged
    1024-col MM fails s3d3_mm_num_elements) into ONE [65,1024] pso tile.
  - Output path in bf16 (PE transpose at 1 cyc/row vs 2 for f32; psum
    slices padded to 66 cols for 4B alignment; host casts back to f32).
    Final era's output DMAs alternate sync/gpsimd queues.
  - PSUM (static pools, 16KB/partition): scores 2x[128,1024]f32 (8KB) +
    pso [65,1024]f32 (4KB) + proj/dup/transpose ring 2x(2KB).
  - Tried and REVERTED: DVE fastexp offload (f32 psum reads get no 2x
    mode -> breakeven speed, and error jumps to 1.6e-2 vs the 2e-2 gate);
    q0/q1 exp interleave (new mid-stream serialization, +5us); f16 psum
    scores (matmul output must be f32); 256-col output quarters (3x the
    serial DVE ops in the tail).
"""

import numpy as np

import concourse.bass as bass
import concourse.mybir as mybir
import concourse.tile as tile
from concourse import bacc
from concourse.bass_utils import run_bass_kernel_spmd
from concourse.masks import make_identity

F32 = mybir.dt.float32
BF16 = mybir.dt.bfloat16
F16 = mybir.dt.float16
I16 = mybir.dt.int16
AF = mybir.ActivationFunctionType
ALU = mybir.AluOpType

S = 2048
DIM = 1024
HD = 64
N_CORES = 8
NCH = DIM // 128   # 8 contraction chunks

SHIFT = -2.0                      # exp(s+SHIFT), cancels in softmax


def build_kernel():
    nc = bacc.Bacc("TRN2", target_bir_lowering=False, debug=False, num_devices=N_CORES)

    # weights host-prearranged to [128, chunk, 128] so the DMA is contiguous
    xt_d = nc.dram_tensor("xt", [DIM, S], BF16, kind="ExternalInput").ap()
    wq_d = nc.dram_tensor("wq", [128, NCH, 128], BF16, kind="ExternalInput").ap()
    wkv_d = nc.dram_tensor("wkv", [128, NCH, 128], BF16, kind="ExternalInput").ap()
    b_d = nc.dram_tensor("b", [128, 2], F32, kind="ExternalInput").ap()
    o_d = nc.dram_tensor("o", [2, S, HD], BF16, kind="ExternalOutput").ap()

    with tile.TileContext(nc) as tc:
        with (
            tc.tile_pool(name="const", bufs=1) as const_pool,
            tc.tile_pool(name="persist", bufs=1) as persist_pool,
            tc.tile_pool(name="pt", bufs=20) as pt_pool,
            tc.tile_pool(name="outs", bufs=2) as out_pool,
            tc.tile_pool(name="ps_s", bufs=2, space="PSUM") as ps_s,
            tc.tile_pool(name="ps_o", bufs=1, space="PSUM") as ps_o,
            tc.tile_pool(name="ps_m", bufs=2, space="PSUM") as ps_m,
        ):
            # ---- persistent SBUF ----
            xT = persist_pool.tile([128, NCH, S], BF16)    # 4 MB
            qt_sb = persist_pool.tile([128, S], BF16)      # rows h*64+d
            kv_sb = persist_pool.tile([128, S], BF16)      # 0:64 KT, 64:128 VT
            kt2u = persist_pool.tile([128, S], BF16)       # KT dup at rows 64:128
            v_sb = persist_pool.tile([128, 16, 65], BF16)  # V' chunks + ones col

            # ---- PE warmup: FIRST PE instructions, zero dependencies.
            # Garbage SBUF as weights+moving; results discarded. Purpose is
            # only to start the HAM pstate ramp (~12us to full clock) ASAP
            # and keep the PE busy until block-0 data lands (~9.2us).
            for w in range(2):
                warm = ps_m.tile([64, 256], F32, tag="proj")
                for r in range(6):
                    nc.tensor.matmul(warm[:], qt_sb[0:64, 0:64],
                                     qt_sb[0:64, 0:256],
                                     start=(r == 0), stop=(r == 5),
                                     skip_group_check=True)

            # ---- input DMAs ----
            # Measured queue rates (v3 trace, concurrent): gpsimd ~100 GB/s,
            # sync ~70, scalar ~68; the vector queue is a 4th stream.
            # dma_start only ISSUES (~0.7us on the sequencer); the DGE queue
            # streams in the background with ~2.5us start latency. Critical
            # path: wkv (split across the 2 fastest queues) -> kv quarter 1,
            # wq -> q chain. xT block 0 spread over all 4 queues.
            wq_sb = const_pool.tile([128, NCH, 128], BF16)
            wkv_sb = const_pool.tile([128, NCH, 128], BF16)
            b_sb = const_pool.tile([128, 2], F32)
            nc.scalar.dma_start(b_sb[:], b_d[:])
            nc.gpsimd.dma_start(wkv_sb[:, 0:4, :], wkv_d[:, 0:4, :])
            nc.sync.dma_start(wkv_sb[:, 4:8, :], wkv_d[:, 4:8, :])
            nc.scalar.dma_start(wq_sb[:], wq_d[:])
            bq_sb = b_sb[:, 0:1]
            bkv_sb = b_sb[:, 1:2]

            xt4 = xt_d.rearrange("(g p) s -> p g s", p=128)  # g: 8 chunks
            s0 = slice(0, 512)
            nc.sync.dma_start(xT[:, 0:2, s0], xt4[:, 0:2, s0])
            nc.gpsimd.dma_start(xT[:, 2:6, s0], xt4[:, 2:6, s0])
            nc.scalar.dma_start(xT[:, 6:8, s0], xt4[:, 6:8, s0])
            B0_ORDER = (0, 1, 2, 3, 4, 5, 6, 7)
            for bb in range(1, 4):
                sl = slice(bb * 512, (bb + 1) * 512)
                nc.scalar.dma_start(xT[:, 0:2, sl], xt4[:, 0:2, sl])
                nc.sync.dma_start(xT[:, 2:4, sl], xt4[:, 2:4, sl])
                nc.gpsimd.dma_start(xT[:, 4:8, sl], xt4[:, 4:8, sl])

            # small consts on vector (queues stay clear for weights/xT)
            for kb in range(16):
                nc.vector.memset(v_sb[:, kb, 64:65], 1.0)
            shift_sb = const_pool.tile([128, 1], F32)
            nc.vector.memset(shift_sb[:], SHIFT)

            # ---- identity (gpsimd emits it AFTER its DMA issues; needed
            # only from dup/vtr at ~15us). ident2 rows 64:128 come from the
            # diagonal block of ident via a same-partition DVE copy (v2 used
            # an SBUF->SBUF DMA on the now-busy sync queue).
            ident = const_pool.tile([128, 128], F32)
            make_identity(nc, ident[:])
            ident2 = const_pool.tile([128, 64], BF16)
            nc.vector.tensor_copy(ident2[0:64, :], ident[0:64, 0:64])
            nc.vector.tensor_copy(ident2[64:128, :], ident[64:128, 64:128])
            identb = const_pool.tile([128, 128], BF16)
            nc.vector.tensor_copy(identb[:], ident[:])

            # ---- helpers ----
            def proj_kv(bb, order=tuple(range(NCH)), cols=slice(0, 512)):
                # cols: column sub-range of the block (kb granularity), used
                # to get kb0's K out ~2.5us earlier during the slow-pstate
                # startup
                lo = bb * 512 + cols.start
                sl = slice(lo, bb * 512 + cols.stop)
                n = cols.stop - cols.start
                pskv = ps_m.tile([128, 512], F32, tag="proj")
                for i, c in enumerate(order):
                    nc.tensor.matmul(pskv[:, 0:n], wkv_sb[:, c, :],
                                     xT[:, c, sl],
                                     start=(i == 0), stop=(i == NCH - 1))
                nc.vector.tensor_scalar_add(kv_sb[:, sl], pskv[:, 0:n],
                                            bkv_sb[:])

            def emit_dup(bb, cols=slice(0, 512)):
                # kt2u dup: col-tiled PE matmul (I64 @ K -> partitions
                # 64:128) + DVE copy -- the DMA queues are saturated with xT.
                sl = slice(bb * 512 + cols.start, bb * 512 + cols.stop)
                n = cols.stop - cols.start
                psd = ps_m.tile([128, 512], F32, tag="proj")
                nc.tensor.matmul(psd[64:128, 0:n], ident2[0:64, :],
                                 kv_sb[0:64, sl], start=True, stop=True)
                nc.vector.tensor_copy(kt2u[64:128, sl], psd[64:128, 0:n])

            psq_pend = {}  # bb -> partially accumulated psq tile

            def proj_q(bb, order=tuple(range(NCH)), part=None):
                # part=0/1 emits one 4-MM half-burst (kept under the score
                # ring's ~1.2us absorption so the exp stream never stalls);
                # part=None emits the whole projection.
                sl = slice(bb * 512, (bb + 1) * 512)
                if part == 1:
                    psq = psq_pend.pop(bb)
                else:
                    psq = ps_m.tile([128, 512], F32, tag="proj")
                cs = order if part is None else order[part * 4:part * 4 + 4]
                for i, c in enumerate(cs):
                    first = (part != 1) and i == 0
                    last = (part != 0) and i == len(cs) - 1
                    nc.tensor.matmul(psq[:], wq_sb[:, c, :], xT[:, c, sl],
                                     start=first, stop=last,
                                     skip_group_check=True)
                if part == 0:
                    psq_pend[bb] = psq
                else:
                    nc.vector.tensor_scalar_add(qt_sb[:, sl], psq[:], bq_sb[:])

            def vtr1(kb):
                ps = ps_m.tile([128, 64], BF16, tag="proj")
                nc.tensor.matmul(
                    ps[:], kv_sb[64:128, kb * 128:(kb + 1) * 128],
                    ident2[64:128, :], is_transpose=True)
                nc.vector.tensor_copy(v_sb[:, kb, 0:64], ps[:])

            def emit_scores(qsl, kb):
                """score pair for (h0,h1) at k-block kb -> [128,1024] psum."""
                pss = ps_s.tile([128, 1024], F32, tag="s")
                kcols = slice(kb * 128, (kb + 1) * 128)
                nc.tensor.matmul(pss[:, 0:512], kv_sb[0:64, kcols],
                                 qt_sb[0:64, qsl], start=True, stop=True)
                nc.tensor.matmul(pss[:, 512:1024], kt2u[64:128, kcols],
                                 qt_sb[64:128, qsl], start=True, stop=True)
                return pss

            def emit_exp(pss, qb, kb, split=False):
                # split=True: h0's half exps as soon as its score MM (gated
                # only by the kv bias) lands, without waiting h1's dup chain
                pt = pt_pool.tile([128, 1024], F16)
                if split:
                    nc.scalar.activation(pt[:, 0:512], pss[:, 0:512],
                                         AF.Exp, bias=shift_sb[:])
                    nc.scalar.activation(pt[:, 512:1024], pss[:, 512:1024],
                                         AF.Exp, bias=shift_sb[:])
                else:
                    nc.scalar.activation(pt[:], pss[:], AF.Exp,
                                         bias=shift_sb[:])
                return pt

            K_FE = 1024 * 1.4426950408889634  # fp16 fastexp slope
            MAGIC = 15360.0 - 29.0 + SHIFT * 1.4426950408889634 * 1024

            def emit_exp_dve(pss):
                # DVE fastexp (bit-trick): i16 = s*K+MAGIC bitcast to f16.
                # ~1.45us/kb vs scalar 1.147, but runs OFF the bottleneck
                # scalar stream; +-1.5% per-element sawtooth error.
                pt = pt_pool.tile([128, 1024], F16)
                nc.vector.tensor_scalar(
                    pt[:, 0:512].bitcast(I16), pss[:, 0:512], K_FE, MAGIC,
                    ALU.mult, ALU.add)
                nc.vector.tensor_scalar(
                    pt[:, 512:1024].bitcast(I16), pss[:, 512:1024], K_FE,
                    MAGIC, ALU.mult, ALU.add)
                return pt

            # exp slots offloaded to the DVE (shortens the scalar stream).
            # EMPTY: any fastexp slot sets max-err to ~1.7e-2 (a single
            # sawtooth peak on a dominant weight dominates the max metric,
            # count-independent) for a sub-noise ~0.5us gain.
            OFF = set()

            def emit_pv(pso, pt, kb):
                # both heads share V' (same KV head); ISA caps a matmul's
                # moving size at 512 cols, so two MMs into one psum tile
                nc.tensor.matmul(pso[:, 0:512], v_sb[:, kb, :], pt[:, 0:512],
                                 start=(kb == 0), stop=(kb == 15),
                                 skip_group_check=True)
                nc.tensor.matmul(pso[:, 512:1024], v_sb[:, kb, :],
                                 pt[:, 512:1024],
                                 start=(kb == 0), stop=(kb == 15),
                                 skip_group_check=True)

            def emit_output(qb, pso, final=False):
                # ot in bf16: halves the PE transpose cost (1 cyc/row vs 2
                # for f32); adds ~0.4% rounding on numerator+denominator.
                # The final era's output is pipelined in 256-col quarters
                # (DVE cast -> PE transpose -> DVE rcp/mult -> DMA) across
                # both idle queues to shrink the serial tail.
                qsl = slice(qb * 512, (qb + 1) * 512)
                nq = 1
                w = 512 // nq
                nt = w // 128  # transposes per piece
                for h in range(2):
                    for q in range(nq):
                        cl = slice(h * 512 + q * w, h * 512 + (q + 1) * w)
                        ot_sb = out_pool.tile([65, w], BF16,
                                              tag=f"ot{h}{q}n{nq}")
                        nc.vector.tensor_copy(ot_sb[:], pso[:, cl])
                        ps = ps_m.tile([128, nt, 66], BF16, tag="proj")
                        for j in range(nt):
                            nc.tensor.transpose(
                                ps[:, j, 0:65],
                                ot_sb[:, j * 128:(j + 1) * 128],
                                identb[:65, :65])
                        rcp = out_pool.tile([128, nt, 1], F32,
                                            tag=f"rcp{h}{q}n{nq}")
                        nc.vector.reciprocal(rcp[:], ps[:, :, 64:65])
                        o_sb = out_pool.tile([128, nt, HD], BF16,
                                             tag=f"o{h}{q}n{nq}")
                        nc.vector.tensor_tensor(
                            o_sb[:], ps[:, :, 0:64],
                            rcp[:].broadcast_to([128, nt, HD]),
                            mybir.AluOpType.mult)
                        eng = nc.gpsimd if (final and (h + q) % 2) else nc.sync
                        rsl = slice(qsl.start + q * w, qsl.start + (q + 1) * w)
                        eng.dma_start(
                            o_d[h, rsl, :].rearrange("(t j) c -> j t c",
                                                     j=128),
                            o_sb[:])

            # ---- unified deferred-work fifo ----
            # Items: ("pv", pso, pt, kb, ready_seq) and ("out", qb, pso).
            # ALL PVs are deferred into the fifo; the q1..q3 eras drain it
            # adaptively (<=2 PV-pairs per kb, more when backlogged) so the
            # exp stream paces the kernel and the PE never falls behind
            # locally. A PV is only popped once its exp is at least one kb
            # in the past (lag>=1), else the in-order PE FIFO would stall
            # on the activation.
            fifo = []
            nseq = [0]

            def drain(kb, qb):
                budget = 2 if (len(fifo) > 6
                               or (qb == 3 and len(fifo) > 15 - kb)) else 1
                popped = 0
                while fifo and popped < budget:
                    it = fifo[0]
                    if it[0] == "pv":
                        if it[4] > nseq[0] - 1:
                            break  # too fresh: exp still in flight
                        fifo.pop(0)
                        emit_pv(it[1], it[2], it[3])
                        popped += 1
                    else:
                        if popped:
                            break  # output starts a fresh kb slot
                        fifo.pop(0)
                        emit_output(it[1], it[2])
                        popped = 2

            # ---- B(0): kv in kb-quarters so kb0's K is ready before the
            # full q chain; q0 scores start earlier at MID pstate ----
            q0 = slice(0, 512)
            pso = ps_o.tile([65, 1024], F32, tag="o")
            proj_kv(0, B0_ORDER, cols=slice(0, 128))
            emit_dup(0, cols=slice(0, 128))
            vtr1(0)
            proj_q(0, B0_ORDER)
            for qq in range(1, 4):
                proj_kv(0, B0_ORDER, cols=slice(qq * 128, qq * 128 + 128))
                emit_dup(0, cols=slice(qq * 128, qq * 128 + 128))
                vtr1(qq)

            # ---- q0 wave pipeline: scores+exp only; kv(bb)/proj_q(bb)
            # spread through the waves; vtr 1/kb; PVs all into the fifo ----
            for bb in range(4):
                if bb > 0:
                    proj_kv(bb)
                    emit_dup(bb)
                for j, kb in enumerate(range(bb * 4, bb * 4 + 4)):
                    pss = emit_scores(q0, kb)
                    pt = emit_exp(pss, 0, kb, split=(bb > 0 and j == 0))
                    fifo.append(("pv", pso, pt, kb, nseq[0] + 1))
                    nseq[0] += 1
                    if kb >= 4:
                        vtr1(kb)
                    if bb > 0 and j == 1:
                        proj_q(bb, part=0)
                    if bb > 0 and j == 2:
                        proj_q(bb, part=1)
            fifo.append(("out", 0, pso))

            # ---- exp-paced eras q1..q3 ----
            for qb in range(1, 4):
                qsl = slice(qb * 512, (qb + 1) * 512)
                pso = ps_o.tile([65, 1024], F32, tag="o")
                for kb in range(16):
                    pss = emit_scores(qsl, kb)
                    if (qb, kb) in OFF:
                        pt = emit_exp_dve(pss)
                        lag = 2
                    else:
                        # split the final two exps: the tail's PV h0 + cast
                        # can start as soon as the h0 half lands
                        pt = emit_exp(pss, qb, kb,
                                      split=(qb == 3 and kb >= 14))
                        lag = 1
                    fifo.append(("pv", pso, pt, kb, nseq[0] + lag))
                    nseq[0] += 1
                    drain(kb, qb)
                fifo.append(("out", qb, pso))
            while fifo:
                it = fifo.pop(0)
                if it[0] == "pv":
                    emit_pv(it[1], it[2], it[3])
                else:
                    emit_output(it[1], it[2], final=(not fifo))

    nc.compile()
    return nc


_NC_CACHE = None


def make_in_maps(inputs):
    import ml_dtypes
    x = np.asarray(inputs["x"], np.float32).reshape(S, DIM)
    xt = np.ascontiguousarray(x.T).astype(ml_dtypes.bfloat16)
    Wq = np.asarray(inputs["Wq"], np.float32)
    bq = np.asarray(inputs["bq"], np.float32)
    Wk = np.asarray(inputs["Wk"], np.float32)
    bk = np.asarray(inputs["bk"], np.float32)
    Wv = np.asarray(inputs["Wv"], np.float32)
    bv = np.asarray(inputs["bv"], np.float32)

    in_maps = []
    for d in range(N_CORES):
        g = d // 2
        wkv = np.concatenate(
            [Wk[:, g * 64:(g + 1) * 64], Wv[:, g * 64:(g + 1) * 64]], axis=1)
        bkv = np.concatenate([bk[g * 64:(g + 1) * 64], bv[g * 64:(g + 1) * 64]])
        wq_s = (Wq[:, d * 128:(d + 1) * 128] / 8.0).astype(ml_dtypes.bfloat16)
        wkv_s = wkv.astype(ml_dtypes.bfloat16)
        b2 = np.stack([bq[d * 128:(d + 1) * 128] / 8.0, bkv], axis=1)
        in_maps.append({
            "xt": xt,
            # [1024,128] -> [128 partition, 8 chunk, 128] contiguous
            "wq": np.ascontiguousarray(wq_s.reshape(NCH, 128, 128).transpose(1, 0, 2)),
            "wkv": np.ascontiguousarray(wkv_s.reshape(NCH, 128, 128).transpose(1, 0, 2)),
            "b": np.ascontiguousarray(b2, dtype=np.float32),
        })
    return in_maps


def kernel(**inputs) -> np.ndarray:
    global _NC_CACHE
    if _NC_CACHE is None:
        _NC_CACHE = build_kernel()
    nc = _NC_CACHE
    in_maps = make_in_maps(inputs)
    res = run_bass_kernel_spmd(nc, in_maps, list(range(N_CORES)))
    blocks = [np.asarray(res.results[d]["o"]).astype(np.float32).reshape(256, DIM)
              for d in range(N_CORES)]
    return np.concatenate(blocks, axis=0).reshape(1, S, DIM).astype(np.float32)


# revision 37
# speedup vs baseline: 1.1475x; 1.1475x over previous
"""GQA kernel for Trainium2 (Bass/Tile), 8-core head-parallel. v4.

Problem: x(1,2048,1024), Wq(1024,1024)+bq, Wk/Wv(1024,256)+bk/bv,
16 Q heads / 4 KV heads, head_dim 64, full (non-causal) softmax attention.
Reference output is attn(B,H,S,Dh) reshaped DIRECTLY to (B,S,H*Dh):
out rows [h*128,(h+1)*128) of the (2048,1024) output belong to head h.

Sharding: core d owns Q heads {2d, 2d+1} (both share KV head d//2), so each
core computes a contiguous (256,1024) slab of the final output.

Host-side prep (free): x transposed+cast to bf16 xT (1024,2048); per-core
weight slices pre-scaled (Wq/8 folds 1/sqrt(64)) and packed Wkv=[Wk|Wv],
all cast to bf16.

v4 structure (v2 baseline 113.6-115us; v4 measures ~112-113us):
  - Engine budget (measured): scalar exp 71us busy (64 ACTIVATEs of 1024
    cols, ~1110ns each), PE union ~85us, DVE ~24us. Both scalar AND PE are
    near-saturated inside the exp window -> all PV/output work is load-
    balanced across the whole exp span via a deferred-work fifo.
  - Timeline: first ACTIVATE ~24-25us (floor: 3-queue DMA lands wkv+wq+xT
    block0 ~15-17us, then kv-quarter1 + q chain at MID pstate), exp gaps
    ~10-12us (all in the q0/wave era), tail ~9us.
  - HAM pstate: PE runs 1.2GHz until ~12-17us of near-continuous activity
    (ham k=8/8 at t~20-26us); idle gaps reset/delay the ramp and can cause
    mid-kernel downclock. Zero-dependency warmup (garbage SBUF weights)
    starts the ramp at ~6.7us. Longer warmup chains DELAY real work ~1:1
    (the Tile DAG scheduler prefers earlier-emitted ready work), so keep
    warmup short.
  - DMA queues (measured concurrent): gpsimd ~100 GB/s, sync ~45-70,
    scalar ~68; the DVE cannot issue DMAs. dma_start only ISSUES; data
    starts ~2.5us later. wkv split across gpsimd+sync first, wq on scalar
    early, xT strips balanced by rate (sync gets only 2-chunk strips).
  - B(0) kv projection in kb-column-quarters so kb0's K is ready before
    the full 512-col q chain finishes.
  - ALL PVs are deferred into a fifo of ("pv", pso, pt, kb, ready_seq) and
    ("out", qb, pso) items, drained <=2 pops per exp slot in the q1..q3
    eras (more when backlogged, none during the DMA-paced q0 waves). Pops
    require the exp >=1 slot old, else the PE stalls on the activation.
    fifo order serializes pso psum-bank reuse across eras (bufs=1 ring).
  - pt ring bufs=20 (~5MB SBUF) holds the deferred exp outputs.
  - PV stays two 512-col MMs per kb (ISA caps matmul moving size; a merged
    1024-col MM fails s3d3_mm_num_elements) into ONE [65,1024] pso tile.
  - Output path in bf16 (PE transpose at 1 cyc/row vs 2 for f32; psum
    slices padded to 66 cols for 4B alignment; host casts back to f32).
    Final era's output DMAs alternate sync/gpsimd queues.
  - PSUM (static pools, 16KB/partition): scores 2x[128,1024]f32 (8KB) +
    pso [65,1024]f32 (4KB) + proj/dup/transpose ring 2x(2KB).
  - Tried and REVERTED: DVE fastexp offload (f32 psum reads get no 2x
    mode -> breakeven speed, and error jumps to 1.6e-2 vs the 2e-2 gate);
    q0/q1 exp interleave (new mid-stream serialization, +5us); f16 psum
    scores (matmul output must be f32); 256-col output quarters (3x the
    serial DVE ops in the tail).
"""

import numpy as np

import concourse.bass as bass
import concourse.mybir as mybir
import concourse.tile as tile
from concourse import bacc
from concourse.bass_utils import run_bass_kernel_spmd
from concourse.masks import make_identity

F32 = mybir.dt.float32
BF16 = mybir.dt.bfloat16
F16 = mybir.dt.float16
I16 = mybir.dt.int16
AF = mybir.ActivationFunctionType
ALU = mybir.AluOpType

S = 2048
DIM = 1024
HD = 64
N_CORES = 8
NCH = DIM // 128   # 8 contraction chunks

SHIFT = -2.0                      # exp(s+SHIFT), cancels in softmax


def build_kernel():
    nc = bacc.Bacc("TRN2", target_bir_lowering=False, debug=False, num_devices=N_CORES)

    # weights host-prearranged to [128, chunk, 128] so the DMA is contiguous
    xt_d = nc.dram_tensor("xt", [DIM, S], BF16, kind="ExternalInput").ap()
    wq_d = nc.dram_tensor("wq", [128, NCH, 128], BF16, kind="ExternalInput").ap()
    wkv_d = nc.dram_tensor("wkv", [128, NCH, 128], BF16, kind="ExternalInput").ap()
    b_d = nc.dram_tensor("b", [128, 2], F32, kind="ExternalInput").ap()
    o_d = nc.dram_tensor("o", [2, S, HD], BF16, kind="ExternalOutput").ap()

    with tile.TileContext(nc) as tc:
        with (
            tc.tile_pool(name="const", bufs=1) as const_pool,
            tc.tile_pool(name="persist", bufs=1) as persist_pool,
            tc.tile_pool(name="pt", bufs=20) as pt_pool,
            tc.tile_pool(name="outs", bufs=2) as out_pool,
            tc.tile_pool(name="ps_s", bufs=2, space="PSUM") as ps_s,
            tc.tile_pool(name="ps_o", bufs=1, space="PSUM") as ps_o,
            tc.tile_pool(name="ps_m", bufs=2, space="PSUM") as ps_m,
        ):
            # ---- persistent SBUF ----
            xT = persist_pool.tile([128, NCH, S], BF16)    # 4 MB
            qt_sb = persist_pool.tile([128, S], BF16)      # rows h*64+d
            kv_sb = persist_pool.tile([128, S], BF16)      # 0:64 KT, 64:128 VT
            kt2u = persist_pool.tile([128, S], BF16)       # KT dup at rows 64:128
            v_sb = persist_pool.tile([128, 16, 65], BF16)  # V' chunks + ones col

            # ---- PE warmup: FIRST PE instructions, zero dependencies.
            # Garbage SBUF as weights+moving; results discarded. Purpose is
            # only to start the HAM pstate ramp (~12us to full clock) ASAP
            # and keep the PE busy until block-0 data lands (~9.2us).
            for w in range(2):
                warm = ps_m.tile([64, 256], F32, tag="proj")
                for r in range(6):
                    nc.tensor.matmul(warm[:], qt_sb[0:64, 0:64],
                                     qt_sb[0:64, 0:256],
                                     start=(r == 0), stop=(r == 5),
                                     skip_group_check=True)

            # ---- input DMAs ----
            # Measured queue rates (v3 trace, concurrent): gpsimd ~100 GB/s,
            # sync ~70, scalar ~68; the vector queue is a 4th stream.
            # dma_start only ISSUES (~0.7us on the sequencer); the DGE queue
            # streams in the background with ~2.5us start latency. Critical
            # path: wkv (split across the 2 fastest queues) -> kv quarter 1,
            # wq -> q chain. xT block 0 spread over all 4 queues.
            wq_sb = const_pool.tile([128, NCH, 128], BF16)
            wkv_sb = const_pool.tile([128, NCH, 128], BF16)
            b_sb = const_pool.tile([128, 2], F32)
            nc.scalar.dma_start(b_sb[:], b_d[:])
            nc.gpsimd.dma_start(wkv_sb[:, 0:4, :], wkv_d[:, 0:4, :])
            nc.sync.dma_start(wkv_sb[:, 4:8, :], wkv_d[:, 4:8, :])
            nc.scalar.dma_start(wq_sb[:], wq_d[:])
            bq_sb = b_sb[:, 0:1]
            bkv_sb = b_sb[:, 1:2]

            xt4 = xt_d.rearrange("(g p) s -> p g s", p=128)  # g: 8 chunks
            s0 = slice(0, 512)
            nc.sync.dma_start(xT[:, 0:2, s0], xt4[:, 0:2, s0])
            nc.gpsimd.dma_start(xT[:, 2:6, s0], xt4[:, 2:6, s0])
            nc.scalar.dma_start(xT[:, 6:8, s0], xt4[:, 6:8, s0])
            B0_ORDER = (0, 1, 2, 3, 4, 5, 6, 7)
            for bb in range(1, 4):
                sl = slice(bb * 512, (bb + 1) * 512)
                nc.scalar.dma_start(xT[:, 0:2, sl], xt4[:, 0:2, sl])
                nc.sync.dma_start(xT[:, 2:4, sl], xt4[:, 2:4, sl])
                nc.gpsimd.dma_start(xT[:, 4:8, sl], xt4[:, 4:8, sl])

            # small consts on vector (queues stay clear for weights/xT)
            for kb in range(16):
                nc.vector.memset(v_sb[:, kb, 64:65], 1.0)
            shift_sb = const_pool.tile([128, 1], F32)
            nc.vector.memset(shift_sb[:], SHIFT)

            # ---- identity (gpsimd emits it AFTER its DMA issues; needed
            # only from dup/vtr at ~15us). ident2 rows 64:128 come from the
            # diagonal block of ident via a same-partition DVE copy (v2 used
            # an SBUF->SBUF DMA on the now-busy sync queue).
            ident = const_pool.tile([128, 128], F32)
            make_identity(nc, ident[:])
            ident2 = const_pool.tile([128, 64], BF16)
            nc.vector.tensor_copy(ident2[0:64, :], ident[0:64, 0:64])
            nc.vector.tensor_copy(ident2[64:128, :], ident[64:128, 64:128])
            identb = const_pool.tile([128, 128], BF16)
            nc.vector.tensor_copy(identb[:], ident[:])

            # ---- helpers ----
            def proj_kv(bb, order=tuple(range(NCH)), cols=slice(0, 512)):
                # cols: column sub-range of the block (kb granularity), used
                # to get kb0's K out ~2.5us earlier during the slow-pstate
                # startup
                lo = bb * 512 + cols.start
                sl = slice(lo, bb * 512 + cols.stop)
                n = cols.stop - cols.start
                pskv = ps_m.tile([128, 512], F32, tag="proj")
                for i, c in enumerate(order):
                    nc.tensor.matmul(pskv[:, 0:n], wkv_sb[:, c, :],
                                     xT[:, c, sl],
                                     start=(i == 0), stop=(i == NCH - 1))
                nc.vector.tensor_scalar_add(kv_sb[:, sl], pskv[:, 0:n],
                                            bkv_sb[:])

            def emit_dup(bb, cols=slice(0, 512)):
                # kt2u dup: col-tiled PE matmul (I64 @ K -> partitions
                # 64:128) + DVE copy -- the DMA queues are saturated with xT.
                sl = slice(bb * 512 + cols.start, bb * 512 + cols.stop)
                n = cols.stop - cols.start
                psd = ps_m.tile([128, 512], F32, tag="proj")
                nc.tensor.matmul(psd[64:128, 0:n], ident2[0:64, :],
                                 kv_sb[0:64, sl], start=True, stop=True)
                nc.vector.tensor_copy(kt2u[64:128, sl], psd[64:128, 0:n])

            psq_pend = {}  # bb -> partially accumulated psq tile

            def proj_q(bb, order=tuple(range(NCH)), part=None):
                # part=0/1 emits one 4-MM half-burst (kept under the score
                # ring's ~1.2us absorption so the exp stream never stalls);
                # part=None emits the whole projection.
                sl = slice(bb * 512, (bb + 1) * 512)
                if part == 1:
                    psq = psq_pend.pop(bb)
                else:
                    psq = ps_m.tile([128, 512], F32, tag="proj")
                cs = order if part is None else order[part * 4:part * 4 + 4]
                for i, c in enumerate(cs):
                    first = (part != 1) and i == 0
                    last = (part != 0) and i == len(cs) - 1
                    nc.tensor.matmul(psq[:], wq_sb[:, c, :], xT[:, c, sl],
                                     start=first, stop=last,
                                     skip_group_check=True)
                if part == 0:
                    psq_pend[bb] = psq
                else:
                    nc.vector.tensor_scalar_add(qt_sb[:, sl], psq[:], bq_sb[:])

            def vtr1(kb):
                ps = ps_m.tile([128, 64], BF16, tag="proj")
                nc.tensor.matmul(
                    ps[:], kv_sb[64:128, kb * 128:(kb + 1) * 128],
                    ident2[64:128, :], is_transpose=True)
                nc.vector.tensor_copy(v_sb[:, kb, 0:64], ps[:])

            def emit_scores(qsl, kb):
                """score pair for (h0,h1) at k-block kb -> [128,1024] psum."""
                pss = ps_s.tile([128, 1024], F32, tag="s")
                kcols = slice(kb * 128, (kb + 1) * 128)
                nc.tensor.matmul(pss[:, 0:512], kv_sb[0:64, kcols],
                                 qt_sb[0:64, qsl], start=True, stop=True)
                nc.tensor.matmul(pss[:, 512:1024], kt2u[64:128, kcols],
                                 qt_sb[64:128, qsl], start=True, stop=True)
                return pss

            def emit_exp(pss, qb, kb, split=False):
                # split=True: h0's half exps as soon as its score MM (gated
                # only by the kv bias) lands, without waiting h1's dup chain
                pt = pt_pool.tile([128, 1024], F16)
                if split:
                    nc.scalar.activation(pt[:, 0:512], pss[:, 0:512],
                                         AF.Exp, bias=shift_sb[:])
                    nc.scalar.activation(pt[:, 512:1024], pss[:, 512:1024],
                                         AF.Exp, bias=shift_sb[:])
                else:
                    nc.scalar.activation(pt[:], pss[:], AF.Exp,
                                         bias=shift_sb[:])
                return pt

            K_FE = 1024 * 1.4426950408889634  # fp16 fastexp slope
            MAGIC = 15360.0 - 29.0 + SHIFT * 1.4426950408889634 * 1024

            def emit_exp_dve(pss):
                # DVE fastexp (bit-trick): i16 = s*K+MAGIC bitcast to f16.
                # ~1.45us/kb vs scalar 1.147, but runs OFF the bottleneck
                # scalar stream; +-1.5% per-element sawtooth error.
                pt = pt_pool.tile([128, 1024], F16)
                nc.vector.tensor_scalar(
                    pt[:, 0:512].bitcast(I16), pss[:, 0:512], K_FE, MAGIC,
                    ALU.mult, ALU.add)
                nc.vector.tensor_scalar(
                    pt[:, 512:1024].bitcast(I16), pss[:, 512:1024], K_FE,
                    MAGIC, ALU.mult, ALU.add)
                return pt

            # exp slots offloaded to the DVE (shortens the scalar stream).
            # EMPTY: any fastexp slot sets max-err to ~1.7e-2 (a single
            # sawtooth peak on a dominant weight dominates the max metric,
            # count-independent) for a sub-noise ~0.5us gain.
            OFF = set()

            def emit_pv(pso, pt, kb):
                # both heads share V' (same KV head); ISA caps a matmul's
                # moving size at 512 cols, so two MMs into one psum tile
                nc.tensor.matmul(pso[:, 0:512], v_sb[:, kb, :], pt[:, 0:512],
                                 start=(kb == 0), stop=(kb == 15),
                                 skip_group_check=True)
                nc.tensor.matmul(pso[:, 512:1024], v_sb[:, kb, :],
                                 pt[:, 512:1024],
                                 start=(kb == 0), stop=(kb == 15),
                                 skip_group_check=True)

            def emit_output(qb, pso, final=False):
                # ot in bf16: halves the PE transpose cost (1 cyc/row vs 2
                # for f32); adds ~0.4% rounding on numerator+denominator.
                # The final era's output is pipelined in 256-col quarters
                # (DVE cast -> PE transpose -> DVE rcp/mult -> DMA) across
                # both idle queues to shrink the serial tail.
                qsl = slice(qb * 512, (qb + 1) * 512)
                nq = 1
                w = 512 // nq
                nt = w // 128  # transposes per piece
                for h in range(2):
                    for q in range(nq):
                        cl = slice(h * 512 + q * w, h * 512 + (q + 1) * w)
                        ot_sb = out_pool.tile([65, w], BF16,
                                              tag=f"ot{h}{q}n{nq}")
                        nc.vector.tensor_copy(ot_sb[:], pso[:, cl])
                        ps = ps_m.tile([128, nt, 66], BF16, tag="proj")
                        for j in range(nt):
                            nc.tensor.transpose(
                                ps[:, j, 0:65],
                                ot_sb[:, j * 128:(j + 1) * 128],
                                identb[:65, :65])
                        rcp = out_pool.tile([128, nt, 1], F32,
                                            tag=f"rcp{h}{q}n{nq}")
                        nc.vector.reciprocal(rcp[:], ps[:, :, 64:65])
                        o_sb = out_pool.tile([128, nt, HD], BF16,
                                             tag=f"o{h}{q}n{nq}")
                        nc.vector.tensor_tensor(
                            o_sb[:], ps[:, :, 0:64],
                            rcp[:].broadcast_to([128, nt, HD]),
                            mybir.AluOpType.mult)
                        eng = nc.gpsimd if (final and (h + q) % 2) else nc.sync
                        rsl = slice(qsl.start + q * w, qsl.start + (q + 1) * w)
                        eng.dma_start(
                            o_d[h, rsl, :].rearrange("(t j) c -> j t c",
                                                     j=128),
                            o_sb[:])

            # ---- unified deferred-work fifo ----
            # Items: ("pv", pso, pt, kb, ready_seq) and ("out", qb, pso).
            # ALL PVs are deferred into the fifo; the q1..q3 eras drain it
            # adaptively (<=2 PV-pairs per kb, more when backlogged) so the
            # exp stream paces the kernel and the PE never falls behind
            # locally. A PV is only popped once its exp is at least one kb
            # in the past (lag>=1), else the in-order PE FIFO would stall
            # on the activation.
            fifo = []
            nseq = [0]

            def drain(kb, qb):
                budget = 2 if (len(fifo) > 6
                               or (qb == 3 and len(fifo) > 15 - kb)) else 1
                popped = 0
                while fifo and popped < budget:
                    it = fifo[0]
                    if it[0] == "pv":
                        if it[4] > nseq[0] - 1:
                            break  # too fresh: exp still in flight
                        fifo.pop(0)
                        emit_pv(it[1], it[2], it[3])
                        popped += 1
                    else:
                        if popped:
                            break  # output starts a fresh kb slot
                        fifo.pop(0)
                        emit_output(it[1], it[2])
                        popped = 2

            # ---- B(0): kv in kb-quarters so kb0's K is ready before the
            # full q chain; q0 scores start earlier at MID pstate ----
            q0 = slice(0, 512)
            pso = ps_o.tile([65, 1024], F32, tag="o")
            proj_kv(0, B0_ORDER, cols=slice(0, 128))
            emit_dup(0, cols=slice(0, 128))
            vtr1(0)
            proj_q(0, B0_ORDER)
            for qq in range(1, 4):
                proj_kv(0, B0_ORDER, cols=slice(qq * 128, qq * 128 + 128))
                emit_dup(0, cols=slice(qq * 128, qq * 128 + 128))
                vtr1(qq)

            # ---- q0 wave pipeline: scores+exp only; kv(bb)/proj_q(bb)
            # spread through the waves; vtr 1/kb; PVs all into the fifo ----
            for bb in range(4):
                if bb > 0:
                    proj_kv(bb)
                    emit_dup(bb)
                for j, kb in enumerate(range(bb * 4, bb * 4 + 4)):
                    pss = emit_scores(q0, kb)
                    pt = emit_exp(pss, 0, kb, split=(bb > 0 and j == 0))
                    fifo.append(("pv", pso, pt, kb, nseq[0] + 1))
                    nseq[0] += 1
                    if kb >= 4:
                        vtr1(kb)
                    if bb > 0 and j == 1:
                        proj_q(bb, part=0)
                    if bb > 0 and j == 2:
                        proj_q(bb, part=1)
            fifo.append(("out", 0, pso))

            # ---- exp-paced eras q1..q3 ----
            for qb in range(1, 4):
                qsl = slice(qb * 512, (qb + 1) * 512)
                pso = ps_o.tile([65, 1024], F32, tag="o")
                for kb in range(16):
                    pss = emit_scores(qsl, kb)
                    if (qb, kb) in OFF:
                        pt = emit_exp_dve(pss)
                        lag = 2
                    else:
                        pt = emit_exp(pss, qb, kb)
                        lag = 1
                    fifo.append(("pv", pso, pt, kb, nseq[0] + lag))
                    nseq[0] += 1
                    drain(kb, qb)
                fifo.append(("out", qb, pso))
            while fifo:
                it = fifo.pop(0)
                if it[0] == "pv":
                    emit_pv(it[1], it[2], it[3])
                else:
                    emit_output(it[1], it[2], final=(not fifo))

    nc.compile()
    return nc


_NC_CACHE = None


def make_in_maps(inputs):
    import ml_dtypes
    x = np.asarray(inputs["x"], np.float32).reshape(S, DIM)
    xt = np.ascontiguousarray(x.T).astype(ml_dtypes.bfloat16)
    Wq = np.asarray(inputs["Wq"], np.float32)
    bq = np.asarray(inputs["bq"], np.float32)
    Wk = np.asarray(inputs["Wk"], np.float32)
    bk = np.asarray(inputs["bk"], np.float32)
    Wv = np.asarray(inputs["Wv"], np.float32)
    bv = np.asarray(inputs["bv"], np.float32)

    in_maps = []
    for d in range(N_CORES):
        g = d // 2
        wkv = np.concatenate(
            [Wk[:, g * 64:(g + 1) * 64], Wv[:, g * 64:(g + 1) * 64]], axis=1)
        bkv = np.concatenate([bk[g * 64:(g + 1) * 64], bv[g * 64:(g + 1) * 64]])
        wq_s = (Wq[:, d * 128:(d + 1) * 128] / 8.0).astype(ml_dtypes.bfloat16)
        wkv_s = wkv.astype(ml_dtypes.bfloat16)
        b2 = np.stack([bq[d * 128:(d + 1) * 128] / 8.0, bkv], axis=1)
        in_maps.append({
            "xt": xt,
            # [1024,128] -> [128 partition, 8 chunk, 128] contiguous
            "wq": np.ascontiguousarray(wq_s.reshape(NCH, 128, 128).transpose(1, 0, 2)),
            "wkv": np.ascontiguousarray(wkv_s.reshape(NCH, 128, 128).transpose(1, 0, 2)),
            "b": np.ascontiguousarray(b2, dtype=np.float32),
        })
    return in_maps


def kernel(**inputs) -> np.ndarray:
    global _NC_CACHE
    if _NC_CACHE is None:
        _NC_CACHE = build_kernel()
    nc = _NC_CACHE
    in_maps = make_in_maps(inputs)
    res = run_bass_kernel_spmd(nc, in_maps, list(range(N_CORES)))
    blocks = [np.asarray(res.results[d]["o"]).astype(np.float32).reshape(256, DIM)
              for d in range(N_CORES)]
    return np.concatenate(blocks, axis=0).reshape(1, S, DIM).astype(np.float32)


# revision 38
# speedup vs baseline: 1.1556x; 1.0070x over previous
"""GQA kernel for Trainium2 (Bass/Tile), 8-core head-parallel. v4.

Problem: x(1,2048,1024), Wq(1024,1024)+bq, Wk/Wv(1024,256)+bk/bv,
16 Q heads / 4 KV heads, head_dim 64, full (non-causal) softmax attention.
Reference output is attn(B,H,S,Dh) reshaped DIRECTLY to (B,S,H*Dh):
out rows [h*128,(h+1)*128) of the (2048,1024) output belong to head h.

Sharding: core d owns Q heads {2d, 2d+1} (both share KV head d//2), so each
core computes a contiguous (256,1024) slab of the final output.

Host-side prep (free): x transposed+cast to bf16 xT (1024,2048); per-core
weight slices pre-scaled (Wq/8 folds 1/sqrt(64)) and packed Wkv=[Wk|Wv],
all cast to bf16.

v4 structure (v2 baseline 113.6-115us; v4 measures ~112-113us):
  - Engine budget (measured): scalar exp 71us busy (64 ACTIVATEs of 1024
    cols, ~1110ns each), PE union ~85us, DVE ~24us. Both scalar AND PE are
    near-saturated inside the exp window -> all PV/output work is load-
    balanced across the whole exp span via a deferred-work fifo.
  - Timeline: first ACTIVATE ~24-25us (floor: 3-queue DMA lands wkv+wq+xT
    block0 ~15-17us, then kv-quarter1 + q chain at MID pstate), exp gaps
    ~10-12us (all in the q0/wave era), tail ~9us.
  - HAM pstate: PE runs 1.2GHz until ~12-17us of near-continuous activity
    (ham k=8/8 at t~20-26us); idle gaps reset/delay the ramp and can cause
    mid-kernel downclock. Zero-dependency warmup (garbage SBUF weights)
    starts the ramp at ~6.7us. Longer warmup chains DELAY real work ~1:1
    (the Tile DAG scheduler prefers earlier-emitted ready work), so keep
    warmup short.
  - DMA queues (measured concurrent): gpsimd ~100 GB/s, sync ~45-70,
    scalar ~68; the DVE cannot issue DMAs. dma_start only ISSUES; data
    starts ~2.5us later. wkv split across gpsimd+sync first, wq on scalar
    early, xT strips balanced by rate (sync gets only 2-chunk strips).
  - B(0) kv projection in kb-column-quarters so kb0's K is ready before
    the full 512-col q chain finishes.
  - ALL PVs are deferred into a fifo of ("pv", pso, pt, kb, ready_seq) and
    ("out", qb, pso) items, drained <=2 pops per exp slot in the q1..q3
    eras (more when backlogged, none during the DMA-paced q0 waves). Pops
    require the exp >=1 slot old, else the PE stalls on the activation.
    fifo order serializes pso psum-bank reuse across eras (bufs=1 ring).
  - pt ring bufs=20 (~5MB SBUF) holds the deferred exp outputs.
  - PV stays two 512-col MMs per kb (ISA caps matmul moving size; a merged
    1024-col MM fails s3d3_mm_num_elements) into ONE [65,1024] pso tile.
  - Output path in bf16 (PE transpose at 1 cyc/row vs 2 for f32; psum
    slices padded to 66 cols for 4B alignment; host casts back to f32).
    Final era's output DMAs alternate sync/gpsimd queues.
  - PSUM (static pools, 16KB/partition): scores 2x[128,1024]f32 (8KB) +
    pso [65,1024]f32 (4KB) + proj/dup/transpose ring 2x(2KB).
  - Tried and REVERTED: DVE fastexp offload (f32 psum reads get no 2x
    mode -> breakeven speed, and error jumps to 1.6e-2 vs the 2e-2 gate);
    q0/q1 exp interleave (new mid-stream serialization, +5us); f16 psum
    scores (matmul output must be f32); 256-col output quarters (3x the
    serial DVE ops in the tail).
"""

import numpy as np

import concourse.bass as bass
import concourse.mybir as mybir
import concourse.tile as tile
from concourse import bacc
from concourse.bass_utils import run_bass_kernel_spmd
from concourse.masks import make_identity

F32 = mybir.dt.float32
BF16 = mybir.dt.bfloat16
F16 = mybir.dt.float16
I16 = mybir.dt.int16
AF = mybir.ActivationFunctionType
ALU = mybir.AluOpType

S = 2048
DIM = 1024
HD = 64
N_CORES = 8
NCH = DIM // 128   # 8 contraction chunks

SHIFT = -2.0                      # exp(s+SHIFT), cancels in softmax


def build_kernel():
    nc = bacc.Bacc("TRN2", target_bir_lowering=False, debug=False, num_devices=N_CORES)

    # weights host-prearranged to [128, chunk, 128] so the DMA is contiguous
    xt_d = nc.dram_tensor("xt", [DIM, S], BF16, kind="ExternalInput").ap()
    wq_d = nc.dram_tensor("wq", [128, NCH, 128], BF16, kind="ExternalInput").ap()
    wkv_d = nc.dram_tensor("wkv", [128, NCH, 128], BF16, kind="ExternalInput").ap()
    b_d = nc.dram_tensor("b", [128, 2], F32, kind="ExternalInput").ap()
    o_d = nc.dram_tensor("o", [2, S, HD], BF16, kind="ExternalOutput").ap()

    with tile.TileContext(nc) as tc:
        with (
            tc.tile_pool(name="const", bufs=1) as const_pool,
            tc.tile_pool(name="persist", bufs=1) as persist_pool,
            tc.tile_pool(name="pt", bufs=20) as pt_pool,
            tc.tile_pool(name="outs", bufs=2) as out_pool,
            tc.tile_pool(name="ps_s", bufs=2, space="PSUM") as ps_s,
            tc.tile_pool(name="ps_o", bufs=1, space="PSUM") as ps_o,
            tc.tile_pool(name="ps_m", bufs=2, space="PSUM") as ps_m,
        ):
            # ---- persistent SBUF ----
            xT = persist_pool.tile([128, NCH, S], BF16)    # 4 MB
            qt_sb = persist_pool.tile([128, S], BF16)      # rows h*64+d
            kv_sb = persist_pool.tile([128, S], BF16)      # 0:64 KT, 64:128 VT
            kt2u = persist_pool.tile([128, S], BF16)       # KT dup at rows 64:128
            v_sb = persist_pool.tile([128, 16, 65], BF16)  # V' chunks + ones col

            # ---- PE warmup: FIRST PE instructions, zero dependencies.
            # Garbage SBUF as weights+moving; results discarded. Purpose is
            # only to start the HAM pstate ramp (~12us to full clock) ASAP
            # and keep the PE busy until block-0 data lands (~9.2us).
            for w in range(2):
                warm = ps_m.tile([64, 256], F32, tag="proj")
                for r in range(6):
                    nc.tensor.matmul(warm[:], qt_sb[0:64, 0:64],
                                     qt_sb[0:64, 0:256],
                                     start=(r == 0), stop=(r == 5),
                                     skip_group_check=True)

            # ---- input DMAs ----
            # Measured queue rates (v3 trace, concurrent): gpsimd ~100 GB/s,
            # sync ~70, scalar ~68; the vector queue is a 4th stream.
            # dma_start only ISSUES (~0.7us on the sequencer); the DGE queue
            # streams in the background with ~2.5us start latency. Critical
            # path: wkv (split across the 2 fastest queues) -> kv quarter 1,
            # wq -> q chain. xT block 0 spread over all 4 queues.
            wq_sb = const_pool.tile([128, NCH, 128], BF16)
            wkv_sb = const_pool.tile([128, NCH, 128], BF16)
            b_sb = const_pool.tile([128, 2], F32)
            nc.scalar.dma_start(b_sb[:], b_d[:])
            nc.gpsimd.dma_start(wkv_sb[:, 0:4, :], wkv_d[:, 0:4, :])
            nc.sync.dma_start(wkv_sb[:, 4:8, :], wkv_d[:, 4:8, :])
            nc.scalar.dma_start(wq_sb[:], wq_d[:])
            bq_sb = b_sb[:, 0:1]
            bkv_sb = b_sb[:, 1:2]

            xt4 = xt_d.rearrange("(g p) s -> p g s", p=128)  # g: 8 chunks
            s0 = slice(0, 512)
            nc.sync.dma_start(xT[:, 0:2, s0], xt4[:, 0:2, s0])
            nc.gpsimd.dma_start(xT[:, 2:6, s0], xt4[:, 2:6, s0])
            nc.scalar.dma_start(xT[:, 6:8, s0], xt4[:, 6:8, s0])
            B0_ORDER = (0, 1, 2, 3, 4, 5, 6, 7)
            for bb in range(1, 4):
                sl = slice(bb * 512, (bb + 1) * 512)
                nc.scalar.dma_start(xT[:, 0:2, sl], xt4[:, 0:2, sl])
                nc.sync.dma_start(xT[:, 2:4, sl], xt4[:, 2:4, sl])
                nc.gpsimd.dma_start(xT[:, 4:8, sl], xt4[:, 4:8, sl])

            # small consts on vector (queues stay clear for weights/xT)
            for kb in range(16):
                nc.vector.memset(v_sb[:, kb, 64:65], 1.0)
            shift_sb = const_pool.tile([128, 1], F32)
            nc.vector.memset(shift_sb[:], SHIFT)

            # ---- identity (gpsimd emits it AFTER its DMA issues; needed
            # only from dup/vtr at ~15us). ident2 rows 64:128 come from the
            # diagonal block of ident via a same-partition DVE copy (v2 used
            # an SBUF->SBUF DMA on the now-busy sync queue).
            ident = const_pool.tile([128, 128], F32)
            make_identity(nc, ident[:])
            ident2 = const_pool.tile([128, 64], BF16)
            nc.vector.tensor_copy(ident2[0:64, :], ident[0:64, 0:64])
            nc.vector.tensor_copy(ident2[64:128, :], ident[64:128, 64:128])
            identb = const_pool.tile([128, 128], BF16)
            nc.vector.tensor_copy(identb[:], ident[:])

            # ---- helpers ----
            def proj_kv(bb, order=tuple(range(NCH)), cols=slice(0, 512)):
                # cols: column sub-range of the block (kb granularity), used
                # to get kb0's K out ~2.5us earlier during the slow-pstate
                # startup
                lo = bb * 512 + cols.start
                sl = slice(lo, bb * 512 + cols.stop)
                n = cols.stop - cols.start
                pskv = ps_m.tile([128, 512], F32, tag="proj")
                for i, c in enumerate(order):
                    nc.tensor.matmul(pskv[:, 0:n], wkv_sb[:, c, :],
                                     xT[:, c, sl],
                                     start=(i == 0), stop=(i == NCH - 1))
                nc.vector.tensor_scalar_add(kv_sb[:, sl], pskv[:, 0:n],
                                            bkv_sb[:])

            def emit_dup(bb, cols=slice(0, 512)):
                # kt2u dup: col-tiled PE matmul (I64 @ K -> partitions
                # 64:128) + DVE copy -- the DMA queues are saturated with xT.
                sl = slice(bb * 512 + cols.start, bb * 512 + cols.stop)
                n = cols.stop - cols.start
                psd = ps_m.tile([128, 512], F32, tag="proj")
                nc.tensor.matmul(psd[64:128, 0:n], ident2[0:64, :],
                                 kv_sb[0:64, sl], start=True, stop=True)
                nc.vector.tensor_copy(kt2u[64:128, sl], psd[64:128, 0:n])

            psq_pend = {}  # bb -> partially accumulated psq tile

            def proj_q(bb, order=tuple(range(NCH)), part=None):
                # part=0/1 emits one 4-MM half-burst (kept under the score
                # ring's ~1.2us absorption so the exp stream never stalls);
                # part=None emits the whole projection.
                sl = slice(bb * 512, (bb + 1) * 512)
                if part == 1:
                    psq = psq_pend.pop(bb)
                else:
                    psq = ps_m.tile([128, 512], F32, tag="proj")
                cs = order if part is None else order[part * 4:part * 4 + 4]
                for i, c in enumerate(cs):
                    first = (part != 1) and i == 0
                    last = (part != 0) and i == len(cs) - 1
                    nc.tensor.matmul(psq[:], wq_sb[:, c, :], xT[:, c, sl],
                                     start=first, stop=last,
                                     skip_group_check=True)
                if part == 0:
                    psq_pend[bb] = psq
                else:
                    nc.vector.tensor_scalar_add(qt_sb[:, sl], psq[:], bq_sb[:])

            def vtr1(kb):
                ps = ps_m.tile([128, 64], BF16, tag="proj")
                nc.tensor.matmul(
                    ps[:], kv_sb[64:128, kb * 128:(kb + 1) * 128],
                    ident2[64:128, :], is_transpose=True)
                nc.vector.tensor_copy(v_sb[:, kb, 0:64], ps[:])

            def emit_scores(qsl, kb):
                """score pair for (h0,h1) at k-block kb -> [128,1024] psum."""
                pss = ps_s.tile([128, 1024], F32, tag="s")
                kcols = slice(kb * 128, (kb + 1) * 128)
                nc.tensor.matmul(pss[:, 0:512], kv_sb[0:64, kcols],
                                 qt_sb[0:64, qsl], start=True, stop=True)
                nc.tensor.matmul(pss[:, 512:1024], kt2u[64:128, kcols],
                                 qt_sb[64:128, qsl], start=True, stop=True)
                return pss

            def emit_exp(pss, qb, kb, split=False):
                # split=True: h0's half exps as soon as its score MM (gated
                # only by the kv bias) lands, without waiting h1's dup chain
                pt = pt_pool.tile([128, 1024], F16)
                if split:
                    nc.scalar.activation(pt[:, 0:512], pss[:, 0:512],
                                         AF.Exp, bias=shift_sb[:])
                    nc.scalar.activation(pt[:, 512:1024], pss[:, 512:1024],
                                         AF.Exp, bias=shift_sb[:])
                else:
                    nc.scalar.activation(pt[:], pss[:], AF.Exp,
                                         bias=shift_sb[:])
                return pt

            K_FE = 1024 * 1.4426950408889634  # fp16 fastexp slope
            MAGIC = 15360.0 - 29.0 + SHIFT * 1.4426950408889634 * 1024

            def emit_exp_dve(pss):
                # DVE fastexp (bit-trick): i16 = s*K+MAGIC bitcast to f16.
                # ~1.45us/kb vs scalar 1.147, but runs OFF the bottleneck
                # scalar stream; +-1.5% per-element sawtooth error.
                pt = pt_pool.tile([128, 1024], F16)
                nc.vector.tensor_scalar(
                    pt[:, 0:512].bitcast(I16), pss[:, 0:512], K_FE, MAGIC,
                    ALU.mult, ALU.add)
                nc.vector.tensor_scalar(
                    pt[:, 512:1024].bitcast(I16), pss[:, 512:1024], K_FE,
                    MAGIC, ALU.mult, ALU.add)
                return pt

            # exp slots offloaded to the DVE (shortens the scalar stream).
            # EMPTY: any fastexp slot sets max-err to ~1.7e-2 (a single
            # sawtooth peak on a dominant weight dominates the max metric,
            # count-independent) for a sub-noise ~0.5us gain.
            OFF = set()

            def emit_pv(pso, pt, kb):
                # both heads share V' (same KV head); ISA caps a matmul's
                # moving size at 512 cols, so two MMs into one psum tile
                nc.tensor.matmul(pso[:, 0:512], v_sb[:, kb, :], pt[:, 0:512],
                                 start=(kb == 0), stop=(kb == 15),
                                 skip_group_check=True)
                nc.tensor.matmul(pso[:, 512:1024], v_sb[:, kb, :],
                                 pt[:, 512:1024],
                                 start=(kb == 0), stop=(kb == 15),
                                 skip_group_check=True)

            def emit_output(qb, pso, final=False):
                # ot in bf16: halves the PE transpose cost (1 cyc/row vs 2
                # for f32); adds ~0.4% rounding on numerator+denominator.
                # The final era's output is pipelined in 256-col quarters
                # (DVE cast -> PE transpose -> DVE rcp/mult -> DMA) across
                # both idle queues to shrink the serial tail.
                qsl = slice(qb * 512, (qb + 1) * 512)
                nq = 1
                w = 512 // nq
                nt = w // 128  # transposes per piece
                for h in range(2):
                    for q in range(nq):
                        cl = slice(h * 512 + q * w, h * 512 + (q + 1) * w)
                        ot_sb = out_pool.tile([65, w], BF16,
                                              tag=f"ot{h}{q}n{nq}")
                        nc.vector.tensor_copy(ot_sb[:], pso[:, cl])
                        ps = ps_m.tile([128, nt, 66], BF16, tag="proj")
                        for j in range(nt):
                            nc.tensor.transpose(
                                ps[:, j, 0:65],
                                ot_sb[:, j * 128:(j + 1) * 128],
                                identb[:65, :65])
                        rcp = out_pool.tile([128, nt, 1], F32,
                                            tag=f"rcp{h}{q}n{nq}")
                        nc.vector.reciprocal(rcp[:], ps[:, :, 64:65])
                        o_sb = out_pool.tile([128, nt, HD], BF16,
                                             tag=f"o{h}{q}n{nq}")
                        nc.vector.tensor_tensor(
                            o_sb[:], ps[:, :, 0:64],
                            rcp[:].broadcast_to([128, nt, HD]),
                            mybir.AluOpType.mult)
                        eng = nc.gpsimd if (final and (h + q) % 2) else nc.sync
                        rsl = slice(qsl.start + q * w, qsl.start + (q + 1) * w)
                        eng.dma_start(
                            o_d[h, rsl, :].rearrange("(t j) c -> j t c",
                                                     j=128),
                            o_sb[:])

            # ---- unified deferred-work fifo ----
            # Items: ("pv", pso, pt, kb, ready_seq) and ("out", qb, pso).
            # ALL PVs are deferred into the fifo; the q1..q3 eras drain it
            # adaptively (<=2 PV-pairs per kb, more when backlogged) so the
            # exp stream paces the kernel and the PE never falls behind
            # locally. A PV is only popped once its exp is at least one kb
            # in the past (lag>=1), else the in-order PE FIFO would stall
            # on the activation.
            fifo = []
            nseq = [0]

            def drain(kb, qb):
                budget = 2 if (len(fifo) > 6
                               or (qb == 3 and len(fifo) > 15 - kb)) else 1
                popped = 0
                while fifo and popped < budget:
                    it = fifo[0]
                    if it[0] == "pv":
                        if it[4] > nseq[0] - 1:
                            break  # too fresh: exp still in flight
                        fifo.pop(0)
                        emit_pv(it[1], it[2], it[3])
                        popped += 1
                    else:
                        if popped:
                            break  # output starts a fresh kb slot
                        fifo.pop(0)
                        emit_output(it[1], it[2])
                        popped = 2

            # ---- B(0): kv in kb-quarters so kb0's K is ready before the
            # full q chain; q0 scores start earlier at MID pstate ----
            q0 = slice(0, 512)
            pso = ps_o.tile([65, 1024], F32, tag="o")
            proj_kv(0, B0_ORDER, cols=slice(0, 128))
            emit_dup(0, cols=slice(0, 128))
            vtr1(0)
            proj_q(0, B0_ORDER)
            for qq in range(1, 4):
                proj_kv(0, B0_ORDER, cols=slice(qq * 128, qq * 128 + 128))
                emit_dup(0, cols=slice(qq * 128, qq * 128 + 128))
                vtr1(qq)

            # ---- q0 wave pipeline: scores+exp only; kv(bb)/proj_q(bb)
            # spread through the waves; vtr 1/kb; PVs all into the fifo ----
            for bb in range(4):
                if bb > 0:
                    proj_kv(bb)
                    emit_dup(bb)
                for j, kb in enumerate(range(bb * 4, bb * 4 + 4)):
                    pss = emit_scores(q0, kb)
                    pt = emit_exp(pss, 0, kb, split=(bb > 0 and j == 0))
                    fifo.append(("pv", pso, pt, kb, nseq[0] + 1))
                    nseq[0] += 1
                    if kb >= 4:
                        vtr1(kb)
                    if bb > 0 and j == 1:
                        proj_q(bb, part=0)
                    if bb > 0 and j == 2:
                        proj_q(bb, part=1)
            fifo.append(("out", 0, pso))

            # ---- exp-paced eras q1..q3 ----
            for qb in range(1, 4):
                qsl = slice(qb * 512, (qb + 1) * 512)
                pso = ps_o.tile([65, 1024], F32, tag="o")
                for kb in range(16):
                    pss = emit_scores(qsl, kb)
                    if (qb, kb) in OFF:
                        pt = emit_exp_dve(pss)
                        lag = 2
                    else:
                        # the very last exp splits h0/h1 so the tail's
                        # PV h0 + cast can start half an ACTIVATE earlier
                        pt = emit_exp(pss, qb, kb,
                                      split=(qb == 3 and kb == 15))
                        lag = 1
                    fifo.append(("pv", pso, pt, kb, nseq[0] + lag))
                    nseq[0] += 1
                    drain(kb, qb)
                fifo.append(("out", qb, pso))
            while fifo:
                it = fifo.pop(0)
                if it[0] == "pv":
                    emit_pv(it[1], it[2], it[3])
                else:
                    emit_output(it[1], it[2], final=(not fifo))

    nc.compile()
    return nc


_NC_CACHE = None


def make_in_maps(inputs):
    import ml_dtypes
    x = np.asarray(inputs["x"], np.float32).reshape(S, DIM)
    xt = np.ascontiguousarray(x.T).astype(ml_dtypes.bfloat16)
    Wq = np.asarray(inputs["Wq"], np.float32)
    bq = np.asarray(inputs["bq"], np.float32)
    Wk = np.asarray(inputs["Wk"], np.float32)
    bk = np.asarray(inputs["bk"], np.float32)
    Wv = np.asarray(inputs["Wv"], np.float32)
    bv = np.asarray(inputs["bv"], np.float32)

    in_maps = []
    for d in range(N_CORES):
        g = d // 2
        wkv = np.concatenate(
            [Wk[:, g * 64:(g + 1) * 64], Wv[:, g * 64:(g + 1) * 64]], axis=1)
        bkv = np.concatenate([bk[g * 64:(g + 1) * 64], bv[g * 64:(g + 1) * 64]])
        wq_s = (Wq[:, d * 128:(d + 1) * 128] / 8.0).astype(ml_dtypes.bfloat16)
        wkv_s = wkv.astype(ml_dtypes.bfloat16)
        b2 = np.stack([bq[d * 128:(d + 1) * 128] / 8.0, bkv], axis=1)
        in_maps.append({
            "xt": xt,
            # [1024,128] -> [128 partition, 8 chunk, 128] contiguous
            "wq": np.ascontiguousarray(wq_s.reshape(NCH, 128, 128).transpose(1, 0, 2)),
            "wkv": np.ascontiguousarray(wkv_s.reshape(NCH, 128, 128).transpose(1, 0, 2)),
            "b": np.ascontiguousarray(b2, dtype=np.float32),
        })
    return in_maps


def kernel(**inputs) -> np.ndarray:
    global _NC_CACHE
    if _NC_CACHE is None:
        _NC_CACHE = build_kernel()
    nc = _NC_CACHE
    in_maps = make_in_maps(inputs)
    res = run_bass_kernel_spmd(nc, in_maps, list(range(N_CORES)))
    blocks = [np.asarray(res.results[d]["o"]).astype(np.float32).reshape(256, DIM)
              for d in range(N_CORES)]
    return np.concatenate(blocks, axis=0).reshape(1, S, DIM).astype(np.float32)


# revision 39
# speedup vs baseline: 1.1631x; 1.0065x over previous
"""GQA kernel for Trainium2 (Bass/Tile), 8-core head-parallel. v4.

Problem: x(1,2048,1024), Wq(1024,1024)+bq, Wk/Wv(1024,256)+bk/bv,
16 Q heads / 4 KV heads, head_dim 64, full (non-causal) softmax attention.
Reference output is attn(B,H,S,Dh) reshaped DIRECTLY to (B,S,H*Dh):
out rows [h*128,(h+1)*128) of the (2048,1024) output belong to head h.

Sharding: core d owns Q heads {2d, 2d+1} (both share KV head d//2), so each
core computes a contiguous (256,1024) slab of the final output.

Host-side prep (free): x transposed+cast to bf16 xT (1024,2048); per-core
weight slices pre-scaled (Wq/8 folds 1/sqrt(64)) and packed Wkv=[Wk|Wv],
all cast to bf16.

v4 structure (v2 baseline 113.6-115us; v4 measures ~112-113us):
  - Engine budget (measured): scalar exp 71us busy (64 ACTIVATEs of 1024
    cols, ~1110ns each), PE union ~85us, DVE ~24us. Both scalar AND PE are
    near-saturated inside the exp window -> all PV/output work is load-
    balanced across the whole exp span via a deferred-work fifo.
  - Timeline: first ACTIVATE ~24-25us (floor: 3-queue DMA lands wkv+wq+xT
    block0 ~15-17us, then kv-quarter1 + q chain at MID pstate), exp gaps
    ~10-12us (all in the q0/wave era), tail ~9us.
  - HAM pstate: PE runs 1.2GHz until ~12-17us of near-continuous activity
    (ham k=8/8 at t~20-26us); idle gaps reset/delay the ramp and can cause
    mid-kernel downclock. Zero-dependency warmup (garbage SBUF weights)
    starts the ramp at ~6.7us. Longer warmup chains DELAY real work ~1:1
    (the Tile DAG scheduler prefers earlier-emitted ready work), so keep
    warmup short.
  - DMA queues (measured concurrent): gpsimd ~100 GB/s, sync ~45-70,
    scalar ~68; the DVE cannot issue DMAs. dma_start only ISSUES; data
    starts ~2.5us later. wkv split across gpsimd+sync first, wq on scalar
    early, xT strips balanced by rate (sync gets only 2-chunk strips).
  - B(0) kv projection in kb-column-quarters so kb0's K is ready before
    the full 512-col q chain finishes.
  - ALL PVs are deferred into a fifo of ("pv", pso, pt, kb, ready_seq) and
    ("out", qb, pso) items, drained <=2 pops per exp slot in the q1..q3
    eras (more when backlogged, none during the DMA-paced q0 waves). Pops
    require the exp >=1 slot old, else the PE stalls on the activation.
    fifo order serializes pso psum-bank reuse across eras (bufs=1 ring).
  - pt ring bufs=20 (~5MB SBUF) holds the deferred exp outputs.
  - PV stays two 512-col MMs per kb (ISA caps matmul moving size; a merged
    1024-col MM fails s3d3_mm_num_elements) into ONE [65,1024] pso tile.
  - Output path in bf16 (PE transpose at 1 cyc/row vs 2 for f32; psum
    slices padded to 66 cols for 4B alignment; host casts back to f32).
    Final era's output DMAs alternate sync/gpsimd queues.
  - PSUM (static pools, 16KB/partition): scores 2x[128,1024]f32 (8KB) +
    pso [65,1024]f32 (4KB) + proj/dup/transpose ring 2x(2KB).
  - Tried and REVERTED: DVE fastexp offload (f32 psum reads get no 2x
    mode -> breakeven speed, and error jumps to 1.6e-2 vs the 2e-2 gate);
    q0/q1 exp interleave (new mid-stream serialization, +5us); f16 psum
    scores (matmul output must be f32); 256-col output quarters (3x the
    serial DVE ops in the tail).
"""

import numpy as np

import concourse.bass as bass
import concourse.mybir as mybir
import concourse.tile as tile
from concourse import bacc
from concourse.bass_utils import run_bass_kernel_spmd
from concourse.masks import make_identity

F32 = mybir.dt.float32
BF16 = mybir.dt.bfloat16
F16 = mybir.dt.float16
I16 = mybir.dt.int16
AF = mybir.ActivationFunctionType
ALU = mybir.AluOpType

S = 2048
DIM = 1024
HD = 64
N_CORES = 8
NCH = DIM // 128   # 8 contraction chunks

SHIFT = -2.0                      # exp(s+SHIFT), cancels in softmax


def build_kernel():
    nc = bacc.Bacc("TRN2", target_bir_lowering=False, debug=False, num_devices=N_CORES)

    # weights host-prearranged to [128, chunk, 128] so the DMA is contiguous
    xt_d = nc.dram_tensor("xt", [DIM, S], BF16, kind="ExternalInput").ap()
    wq_d = nc.dram_tensor("wq", [128, NCH, 128], BF16, kind="ExternalInput").ap()
    wkv_d = nc.dram_tensor("wkv", [128, NCH, 128], BF16, kind="ExternalInput").ap()
    b_d = nc.dram_tensor("b", [128, 2], F32, kind="ExternalInput").ap()
    o_d = nc.dram_tensor("o", [2, S, HD], BF16, kind="ExternalOutput").ap()

    with tile.TileContext(nc) as tc:
        with (
            tc.tile_pool(name="const", bufs=1) as const_pool,
            tc.tile_pool(name="persist", bufs=1) as persist_pool,
            tc.tile_pool(name="pt", bufs=20) as pt_pool,
            tc.tile_pool(name="outs", bufs=2) as out_pool,
            tc.tile_pool(name="ps_s", bufs=2, space="PSUM") as ps_s,
            tc.tile_pool(name="ps_o", bufs=1, space="PSUM") as ps_o,
            tc.tile_pool(name="ps_m", bufs=2, space="PSUM") as ps_m,
        ):
            # ---- persistent SBUF ----
            xT = persist_pool.tile([128, NCH, S], BF16)    # 4 MB
            qt_sb = persist_pool.tile([128, S], BF16)      # rows h*64+d
            kv_sb = persist_pool.tile([128, S], BF16)      # 0:64 KT, 64:128 VT
            kt2u = persist_pool.tile([128, S], BF16)       # KT dup at rows 64:128
            v_sb = persist_pool.tile([128, 16, 65], BF16)  # V' chunks + ones col

            # ---- PE warmup: FIRST PE instructions, zero dependencies.
            # Garbage SBUF as weights+moving; results discarded. Purpose is
            # only to start the HAM pstate ramp (~12us to full clock) ASAP
            # and keep the PE busy until block-0 data lands (~9.2us).
            for w in range(2):
                warm = ps_m.tile([64, 256], F32, tag="proj")
                for r in range(6):
                    nc.tensor.matmul(warm[:], qt_sb[0:64, 0:64],
                                     qt_sb[0:64, 0:256],
                                     start=(r == 0), stop=(r == 5),
                                     skip_group_check=True)

            # ---- input DMAs ----
            # Measured queue rates (v3 trace, concurrent): gpsimd ~100 GB/s,
            # sync ~70, scalar ~68; the vector queue is a 4th stream.
            # dma_start only ISSUES (~0.7us on the sequencer); the DGE queue
            # streams in the background with ~2.5us start latency. Critical
            # path: wkv (split across the 2 fastest queues) -> kv quarter 1,
            # wq -> q chain. xT block 0 spread over all 4 queues.
            wq_sb = const_pool.tile([128, NCH, 128], BF16)
            wkv_sb = const_pool.tile([128, NCH, 128], BF16)
            b_sb = const_pool.tile([128, 2], F32)
            nc.scalar.dma_start(b_sb[:], b_d[:])
            nc.gpsimd.dma_start(wkv_sb[:, 0:4, :], wkv_d[:, 0:4, :])
            nc.sync.dma_start(wkv_sb[:, 4:8, :], wkv_d[:, 4:8, :])
            nc.scalar.dma_start(wq_sb[:], wq_d[:])
            bq_sb = b_sb[:, 0:1]
            bkv_sb = b_sb[:, 1:2]

            xt4 = xt_d.rearrange("(g p) s -> p g s", p=128)  # g: 8 chunks
            s0 = slice(0, 512)
            nc.sync.dma_start(xT[:, 0:2, s0], xt4[:, 0:2, s0])
            nc.gpsimd.dma_start(xT[:, 2:6, s0], xt4[:, 2:6, s0])
            nc.scalar.dma_start(xT[:, 6:8, s0], xt4[:, 6:8, s0])
            B0_ORDER = (0, 1, 2, 3, 4, 5, 6, 7)
            for bb in range(1, 4):
                sl = slice(bb * 512, (bb + 1) * 512)
                nc.scalar.dma_start(xT[:, 0:2, sl], xt4[:, 0:2, sl])
                nc.sync.dma_start(xT[:, 2:4, sl], xt4[:, 2:4, sl])
                nc.gpsimd.dma_start(xT[:, 4:8, sl], xt4[:, 4:8, sl])

            # small consts on vector (queues stay clear for weights/xT)
            for kb in range(16):
                nc.vector.memset(v_sb[:, kb, 64:65], 1.0)
            shift_sb = const_pool.tile([128, 1], F32)
            nc.vector.memset(shift_sb[:], SHIFT)

            # ---- identity (gpsimd emits it AFTER its DMA issues; needed
            # only from dup/vtr at ~15us). ident2 rows 64:128 come from the
            # diagonal block of ident via a same-partition DVE copy (v2 used
            # an SBUF->SBUF DMA on the now-busy sync queue).
            ident = const_pool.tile([128, 128], F32)
            make_identity(nc, ident[:])
            ident2 = const_pool.tile([128, 64], BF16)
            nc.vector.tensor_copy(ident2[0:64, :], ident[0:64, 0:64])
            nc.vector.tensor_copy(ident2[64:128, :], ident[64:128, 64:128])
            identb = const_pool.tile([128, 128], BF16)
            nc.vector.tensor_copy(identb[:], ident[:])

            # ---- helpers ----
            def proj_kv(bb, order=tuple(range(NCH)), cols=slice(0, 512),
                        bias_split=False):
                # cols: column sub-range of the block (kb granularity).
                # bias_split: bias the first 128 cols separately so the
                # wave's first score (gated by bias->dup->cast) unblocks
                # ~1us earlier at each wave boundary.
                lo = bb * 512 + cols.start
                sl = slice(lo, bb * 512 + cols.stop)
                n = cols.stop - cols.start
                pskv = ps_m.tile([128, 512], F32, tag="proj")
                for i, c in enumerate(order):
                    nc.tensor.matmul(pskv[:, 0:n], wkv_sb[:, c, :],
                                     xT[:, c, sl],
                                     start=(i == 0), stop=(i == NCH - 1))
                if bias_split:
                    s1 = slice(lo, lo + 128)
                    nc.vector.tensor_scalar_add(kv_sb[:, s1], pskv[:, 0:128],
                                                bkv_sb[:])
                    s2 = slice(lo + 128, bb * 512 + cols.stop)
                    nc.vector.tensor_scalar_add(kv_sb[:, s2],
                                                pskv[:, 128:n], bkv_sb[:])
                else:
                    nc.vector.tensor_scalar_add(kv_sb[:, sl], pskv[:, 0:n],
                                                bkv_sb[:])

            def emit_dup(bb, cols=slice(0, 512)):
                # kt2u dup: col-tiled PE matmul (I64 @ K -> partitions
                # 64:128) + DVE copy -- the DMA queues are saturated with xT.
                sl = slice(bb * 512 + cols.start, bb * 512 + cols.stop)
                n = cols.stop - cols.start
                psd = ps_m.tile([128, 512], F32, tag="proj")
                nc.tensor.matmul(psd[64:128, 0:n], ident2[0:64, :],
                                 kv_sb[0:64, sl], start=True, stop=True)
                nc.vector.tensor_copy(kt2u[64:128, sl], psd[64:128, 0:n])

            psq_pend = {}  # bb -> partially accumulated psq tile

            def proj_q(bb, order=tuple(range(NCH)), part=None):
                # part=0/1 emits one 4-MM half-burst (kept under the score
                # ring's ~1.2us absorption so the exp stream never stalls);
                # part=None emits the whole projection.
                sl = slice(bb * 512, (bb + 1) * 512)
                if part == 1:
                    psq = psq_pend.pop(bb)
                else:
                    psq = ps_m.tile([128, 512], F32, tag="proj")
                cs = order if part is None else order[part * 4:part * 4 + 4]
                for i, c in enumerate(cs):
                    first = (part != 1) and i == 0
                    last = (part != 0) and i == len(cs) - 1
                    nc.tensor.matmul(psq[:], wq_sb[:, c, :], xT[:, c, sl],
                                     start=first, stop=last,
                                     skip_group_check=True)
                if part == 0:
                    psq_pend[bb] = psq
                else:
                    nc.vector.tensor_scalar_add(qt_sb[:, sl], psq[:], bq_sb[:])

            def vtr1(kb):
                ps = ps_m.tile([128, 64], BF16, tag="proj")
                nc.tensor.matmul(
                    ps[:], kv_sb[64:128, kb * 128:(kb + 1) * 128],
                    ident2[64:128, :], is_transpose=True)
                nc.vector.tensor_copy(v_sb[:, kb, 0:64], ps[:])

            def emit_scores(qsl, kb):
                """score pair for (h0,h1) at k-block kb -> [128,1024] psum."""
                pss = ps_s.tile([128, 1024], F32, tag="s")
                kcols = slice(kb * 128, (kb + 1) * 128)
                nc.tensor.matmul(pss[:, 0:512], kv_sb[0:64, kcols],
                                 qt_sb[0:64, qsl], start=True, stop=True)
                nc.tensor.matmul(pss[:, 512:1024], kt2u[64:128, kcols],
                                 qt_sb[64:128, qsl], start=True, stop=True)
                return pss

            def emit_exp(pss, qb, kb, split=False):
                # split=True: h0's half exps as soon as its score MM (gated
                # only by the kv bias) lands, without waiting h1's dup chain
                pt = pt_pool.tile([128, 1024], F16)
                if split:
                    nc.scalar.activation(pt[:, 0:512], pss[:, 0:512],
                                         AF.Exp, bias=shift_sb[:])
                    nc.scalar.activation(pt[:, 512:1024], pss[:, 512:1024],
                                         AF.Exp, bias=shift_sb[:])
                else:
                    nc.scalar.activation(pt[:], pss[:], AF.Exp,
                                         bias=shift_sb[:])
                return pt

            K_FE = 1024 * 1.4426950408889634  # fp16 fastexp slope
            MAGIC = 15360.0 - 29.0 + SHIFT * 1.4426950408889634 * 1024

            def emit_exp_dve(pss):
                # DVE fastexp (bit-trick): i16 = s*K+MAGIC bitcast to f16.
                # ~1.45us/kb vs scalar 1.147, but runs OFF the bottleneck
                # scalar stream; +-1.5% per-element sawtooth error.
                pt = pt_pool.tile([128, 1024], F16)
                nc.vector.tensor_scalar(
                    pt[:, 0:512].bitcast(I16), pss[:, 0:512], K_FE, MAGIC,
                    ALU.mult, ALU.add)
                nc.vector.tensor_scalar(
                    pt[:, 512:1024].bitcast(I16), pss[:, 512:1024], K_FE,
                    MAGIC, ALU.mult, ALU.add)
                return pt

            # exp slots offloaded to the DVE (shortens the scalar stream).
            # EMPTY: any fastexp slot sets max-err to ~1.7e-2 (a single
            # sawtooth peak on a dominant weight dominates the max metric,
            # count-independent) for a sub-noise ~0.5us gain.
            OFF = set()

            def emit_pv(psop, pt, kb):
                # both heads share V' (same KV head); ISA caps a matmul's
                # moving size at 512 cols. Separate per-head psum tiles so
                # h0's output chain never waits h1's last PV.
                nc.tensor.matmul(psop[0][:], v_sb[:, kb, :], pt[:, 0:512],
                                 start=(kb == 0), stop=(kb == 15),
                                 skip_group_check=True)
                nc.tensor.matmul(psop[1][:], v_sb[:, kb, :],
                                 pt[:, 512:1024],
                                 start=(kb == 0), stop=(kb == 15),
                                 skip_group_check=True)

            def emit_output(qb, pso, final=False):
                # ot in bf16: halves the PE transpose cost (1 cyc/row vs 2
                # for f32); adds ~0.4% rounding on numerator+denominator.
                # The final era's output is pipelined in 256-col quarters
                # (DVE cast -> PE transpose -> DVE rcp/mult -> DMA) across
                # both idle queues to shrink the serial tail.
                qsl = slice(qb * 512, (qb + 1) * 512)
                nq = 1
                w = 512 // nq
                nt = w // 128  # transposes per piece
                for h in range(2):
                    for q in range(nq):
                        cl = slice(q * w, (q + 1) * w)
                        ot_sb = out_pool.tile([65, w], BF16,
                                              tag=f"ot{h}{q}n{nq}")
                        nc.vector.tensor_copy(ot_sb[:], pso[h][:, cl])
                        ps = ps_m.tile([128, nt, 66], BF16, tag="proj")
                        for j in range(nt):
                            nc.tensor.transpose(
                                ps[:, j, 0:65],
                                ot_sb[:, j * 128:(j + 1) * 128],
                                identb[:65, :65])
                        rcp = out_pool.tile([128, nt, 1], F32,
                                            tag=f"rcp{h}{q}n{nq}")
                        nc.vector.reciprocal(rcp[:], ps[:, :, 64:65])
                        o_sb = out_pool.tile([128, nt, HD], BF16,
                                             tag=f"o{h}{q}n{nq}")
                        nc.vector.tensor_tensor(
                            o_sb[:], ps[:, :, 0:64],
                            rcp[:].broadcast_to([128, nt, HD]),
                            mybir.AluOpType.mult)
                        eng = nc.gpsimd if (final and (h + q) % 2) else nc.sync
                        rsl = slice(qsl.start + q * w, qsl.start + (q + 1) * w)
                        eng.dma_start(
                            o_d[h, rsl, :].rearrange("(t j) c -> j t c",
                                                     j=128),
                            o_sb[:])

            # ---- unified deferred-work fifo ----
            # Items: ("pv", pso, pt, kb, ready_seq) and ("out", qb, pso).
            # ALL PVs are deferred into the fifo; the q1..q3 eras drain it
            # adaptively (<=2 PV-pairs per kb, more when backlogged) so the
            # exp stream paces the kernel and the PE never falls behind
            # locally. A PV is only popped once its exp is at least one kb
            # in the past (lag>=1), else the in-order PE FIFO would stall
            # on the activation.
            fifo = []
            nseq = [0]

            def drain(kb, qb):
                budget = 2 if (len(fifo) > 6
                               or (qb == 3 and len(fifo) > 15 - kb)) else 1
                popped = 0
                while fifo and popped < budget:
                    it = fifo[0]
                    if it[0] == "pv":
                        if it[4] > nseq[0] - 1:
                            break  # too fresh: exp still in flight
                        fifo.pop(0)
                        emit_pv(it[1], it[2], it[3])
                        popped += 1
                    else:
                        if popped:
                            break  # output starts a fresh kb slot
                        fifo.pop(0)
                        emit_output(it[1], it[2])
                        popped = 2

            # ---- B(0): kv in kb-quarters so kb0's K is ready before the
            # full q chain; q0 scores start earlier at MID pstate ----
            q0 = slice(0, 512)
            poa = ps_o.tile([65, 512], F32, tag="oa")
            pob = ps_o.tile([65, 512], F32, tag="ob")
            pso = (poa, pob)
            proj_kv(0, B0_ORDER, cols=slice(0, 128))
            emit_dup(0, cols=slice(0, 128))
            vtr1(0)
            proj_q(0, B0_ORDER)
            for qq in range(1, 4):
                proj_kv(0, B0_ORDER, cols=slice(qq * 128, qq * 128 + 128))
                emit_dup(0, cols=slice(qq * 128, qq * 128 + 128))
                vtr1(qq)

            # ---- q0 wave pipeline: scores+exp only; kv(bb)/proj_q(bb)
            # spread through the waves; vtr 1/kb; PVs all into the fifo ----
            for bb in range(4):
                if bb > 0:
                    proj_kv(bb, bias_split=True)
                    emit_dup(bb, cols=slice(0, 128))
                    emit_dup(bb, cols=slice(128, 512))
                for j, kb in enumerate(range(bb * 4, bb * 4 + 4)):
                    pss = emit_scores(q0, kb)
                    pt = emit_exp(pss, 0, kb, split=(bb > 0 and j == 0))
                    fifo.append(("pv", pso, pt, kb, nseq[0] + 1))
                    nseq[0] += 1
                    if kb >= 4:
                        vtr1(kb)
                    if bb > 0 and j == 1:
                        proj_q(bb, part=0)
                    if bb > 0 and j == 2:
                        proj_q(bb, part=1)
            fifo.append(("out", 0, pso))

            # ---- exp-paced eras q1..q3 ----
            for qb in range(1, 4):
                qsl = slice(qb * 512, (qb + 1) * 512)
                poa = ps_o.tile([65, 512], F32, tag="oa")
                pob = ps_o.tile([65, 512], F32, tag="ob")
                pso = (poa, pob)
                for kb in range(16):
                    pss = emit_scores(qsl, kb)
                    if (qb, kb) in OFF:
                        pt = emit_exp_dve(pss)
                        lag = 2
                    else:
                        # the very last exp splits h0/h1 so the tail's
                        # PV h0 + cast can start half an ACTIVATE earlier
                        pt = emit_exp(pss, qb, kb,
                                      split=(qb == 3 and kb == 15))
                        lag = 1
                    fifo.append(("pv", pso, pt, kb, nseq[0] + lag))
                    nseq[0] += 1
                    drain(kb, qb)
                fifo.append(("out", qb, pso))
            while fifo:
                it = fifo.pop(0)
                if it[0] == "pv":
                    emit_pv(it[1], it[2], it[3])
                else:
                    emit_output(it[1], it[2], final=(not fifo))

    nc.compile()
    return nc


_NC_CACHE = None


def make_in_maps(inputs):
    import ml_dtypes
    x = np.asarray(inputs["x"], np.float32).reshape(S, DIM)
    xt = np.ascontiguousarray(x.T).astype(ml_dtypes.bfloat16)
    Wq = np.asarray(inputs["Wq"], np.float32)
    bq = np.asarray(inputs["bq"], np.float32)
    Wk = np.asarray(inputs["Wk"], np.float32)
    bk = np.asarray(inputs["bk"], np.float32)
    Wv = np.asarray(inputs["Wv"], np.float32)
    bv = np.asarray(inputs["bv"], np.float32)

    in_maps = []
    for d in range(N_CORES):
        g = d // 2
        wkv = np.concatenate(
            [Wk[:, g * 64:(g + 1) * 64], Wv[:, g * 64:(g + 1) * 64]], axis=1)
        bkv = np.concatenate([bk[g * 64:(g + 1) * 64], bv[g * 64:(g + 1) * 64]])
        wq_s = (Wq[:, d * 128:(d + 1) * 128] / 8.0).astype(ml_dtypes.bfloat16)
        wkv_s = wkv.astype(ml_dtypes.bfloat16)
        b2 = np.stack([bq[d * 128:(d + 1) * 128] / 8.0, bkv], axis=1)
        in_maps.append({
            "xt": xt,
            # [1024,128] -> [128 partition, 8 chunk, 128] contiguous
            "wq": np.ascontiguousarray(wq_s.reshape(NCH, 128, 128).transpose(1, 0, 2)),
            "wkv": np.ascontiguousarray(wkv_s.reshape(NCH, 128, 128).transpose(1, 0, 2)),
            "b": np.ascontiguousarray(b2, dtype=np.float32),
        })
    return in_maps


def kernel(**inputs) -> np.ndarray:
    global _NC_CACHE
    if _NC_CACHE is None:
        _NC_CACHE = build_kernel()
    nc = _NC_CACHE
    in_maps = make_in_maps(inputs)
    res = run_bass_kernel_spmd(nc, in_maps, list(range(N_CORES)))
    blocks = [np.asarray(res.results[d]["o"]).astype(np.float32).reshape(256, DIM)
              for d in range(N_CORES)]
    return np.concatenate(blocks, axis=0).reshape(1, S, DIM).astype(np.float32)
